# revision 16
# baseline (speedup 1.0000x reference)
"""Trainium2 Bass kernel for EnhancedInvariantExtractor — v4 (bf16, tiled PE).

Input  h [1_000_000, 120] f32:  per atom: 32 scalars | 16 vectors (l=1, dim 3)
                                | 8 tensors (l=2, dim 5).
Output [1_000_000, 204] f32: scalars(32) | vnorm(16) | tnorm(8) | vdots(120)
                             | tdots(28).

Strategy (8 NeuronCores, data-parallel over atoms):
- Scalars are pure passthrough -> copied host-side, never shipped.
- Everything device-side is bf16 (fp16 moving data runs the PE at half rate).
- HW rule discovered by bisection: PSUM accumulation across matmul
  instructions is only safe when all contributors share the same
  tile_position + footprint; cross-row-group accumulation wedges the device.
  The whole design below is single-writer except one same-tp K=32 chain.
- Input host-transposed to d-major [112, padded], t_d4 replicated into every
  32-strip so any consumer finds it in its own row-group:
    s0: vx(16) | t_d0(8) | t_d4 copy(8)
    s1: vy(16) | t_d1(8) | t_d4(8)
    s2: vz(16) | t_d2(8) | t_d4 copy(8)
    s3: t_d3(8) | t_d4 copy(8)            (112 rows)
- u/squ banks (PSUM), pair-group-major so mm4 needs no accumulation:
    V-bank k (k=0..3): strips 0..2 = the 3 vec components of pair group gk
    (strip s holds component (s-k)%3 -> mm3 cells spread), strip 3 = t_dk
    component of all 28 tens pairs (+4 zero pad).
    T-bank: tens d4 pair sums at partitions 96:124 (so the mm4 tens chain
    shares tile_position (96, *) with the V-bank strip-3 contributions).
- Per chunk (512 atoms):
    sq = x*x (gpsimd) ; n2 = S1^T sq (PE, 4 chunks pack one PSUM bank)
    ln/exp -> rinv (ACT bf16) ; norm = n2*rinv (DVE bf16, per group)
    rexp = E4^T rinv (PE, bank shared with F) ; vu = x*rexp (DVE bf16)
    mm3: 17 single-writer tile_position matmuls -> V0..V3, T
    squares: ACT Square on the two 2-bank tiles [128,1024]; T-bank d4 via
      DVE copy (PSUM->SBUF bf16 at partitions 96:128) + gpsimd square
    mm4 vec: one K=96 matmul per F strip (T,T, no accumulation)
    mm4 tens: 5-instr same-tp chain K=32 @ (96, 32j) -> G strip j (4-chunk
      packed; all 28 tens dots)
    evacF = F - 1 (DVE tensor_scalar -> bf16) ; evacG per group (DVE).
- Outputs: outD [128, padded] (vdots 0:120), outG/outN [128, padded/4]
  4-chunk packed. Host deinterleaves + upcasts (free).
- ACT table pinned to 'natural_log_exp_and_others' (Ln+Exp+Square+Copy).
"""

import os
import sys

sys.path.insert(0, "/opt/trn_rl_repo")

import numpy as np
import ml_dtypes

BF16 = ml_dtypes.bfloat16

N_ATOMS = 1_000_000
N_CORES = 8
PER_CORE = N_ATOMS // N_CORES  # 125_000
CHUNK = 512
N_CHUNKS = 245
PADDED = CHUNK * N_CHUNKS  # 125_440
NIN = 112
NOUT = 204
NV, NT = 16, 8
EPS2 = 1e-12

_CACHE = {}

# ---------------------------------------------------------------------------
# layout tables
# ---------------------------------------------------------------------------

_VI, _VJ = np.triu_indices(NV, k=1)  # 120 vec pairs, reference order
_TI, _TJ = np.triu_indices(NT, k=1)  # 28 tens pairs

# tens component d -> input row base, per strip copies for d4
_TBASE = {0: 16, 1: 48, 2: 80, 3: 96}
_T4BASE = {0: 24, 1: 56, 2: 88, 3: 104}  # d4 copy per strip


def _in_rowmap():
    rows = np.zeros(NIN, np.int64)
    for i in range(NV):
        rows[i] = 32 + 3 * i + 0  # vx
        rows[32 + i] = 32 + 3 * i + 1  # vy
        rows[64 + i] = 32 + 3 * i + 2  # vz
    for a in range(NT):
        for d in range(4):
            rows[_TBASE[d] + a] = 80 + 5 * a + d
        for s in range(4):
            rows[_T4BASE[s] + a] = 80 + 5 * a + 4
    return rows


def _vrow(d, i):
    return 32 * d + i


# vec pair groups: bank k holds pairs _G[k]
_GB = [(0, 32), (32, 64), (64, 96), (96, 120)]


def _bank_strip(bank, strip):
    """slot list (32 entries) for V-bank `bank`, strip `strip`.

    entries: ("v", d, p) vec pair p component d | ("t", d, p) tens pair p
    component d | None.
    """
    if strip < 3:
        d = (strip - bank) % 3
        p0, p1 = _GB[bank]
        s = [("v", d, p) for p in range(p0, p1)]
        return s + [None] * (32 - len(s))
    else:
        return [("t", bank, p) for p in range(28)] + [None] * 4


def _t_strip():
    """T-bank partitions 96:128 slot list: d4 components."""
    return [("t", 4, p) for p in range(28)] + [None] * 4


def _slot_src(ent, strip_rg):
    """input/vu rows feeding a u-slot entry, sourced from row-group strip_rg."""
    kind, d, p = ent
    if kind == "v":
        i, j = _VI[p], _VJ[p]
        assert d == strip_rg % 3 or True
        return [_vrow(d, i), _vrow(d, j)]
    a, b = _TI[p], _TJ[p]
    base = _T4BASE[strip_rg] if d == 4 else _TBASE[d]
    return [base + a, base + b]


def _build_consts():
    """Build the packed constant tile [128, W] and the matmul plans."""
    W = 1152
    CM = np.zeros((128, W), np.float32)
    col = [0]

    def alloc(M):
        c0 = col[0]
        col[0] += M
        return c0

    # --- mm1: S1 [112, 32] (primary rows only, no d4 copies) ---
    s1_c0 = alloc(32)
    for i in range(NV):
        for r in (_vrow(0, i), _vrow(1, i), _vrow(2, i)):
            CM[r, s1_c0 + i] = 1.0
    for a in range(NT):
        for d in range(4):
            CM[_TBASE[d] + a, s1_c0 + 16 + a] = 1.0
        CM[_T4BASE[1] + a, s1_c0 + 16 + a] = 1.0
    s1_ref = (0, s1_c0, NIN, 32)

    # --- mm2: E4 [24, 112] x4 at rows 32j (fills all d4 copies) ---
    e4_c0 = alloc(NIN)
    for j in range(4):
        r0 = 32 * j
        for i in range(NV):
            for rr in (_vrow(0, i), _vrow(1, i), _vrow(2, i)):
                CM[r0 + i, e4_c0 + rr] = 1.0
        for a in range(NT):
            for d in range(4):
                CM[r0 + 16 + a, e4_c0 + _TBASE[d] + a] = 1.0
            for s in range(4):
                CM[r0 + 16 + a, e4_c0 + _T4BASE[s] + a] = 1.0
    e4_refs = [(32 * j, e4_c0, 24, NIN) for j in range(4)]

    # --- mm3: all single-writer [*,32] tiles, T/T ---
    plans3 = []
    for bank in range(4):
        for strip in range(4):
            lay = _bank_strip(bank, strip)
            rg = ((strip - bank) % 3) if strip < 3 else bank
            K = 16 if rg == 3 else 32  # input strip 3 has 16 rows
            c0 = alloc(32)
            for s, ent in enumerate(lay):
                if ent is None:
                    continue
                for src in _slot_src(ent, rg):
                    assert 32 * rg <= src < 32 * rg + K, (bank, strip, ent)
                    CM[src, c0 + s] += 1.0
            plans3.append(dict(
                stat=(32 * rg, c0, K, 32), mov=32 * rg, K=K,
                out=("V", bank, 32 * strip, 32), tp=(32 * rg, 32 * strip),
                start=True, stop=True, tag=f"mm3V{bank}s{strip}",
            ))
    # d4 -> T bank partitions 96:128, sourced from primary rows (rg1)
    c0 = alloc(32)
    for s, ent in enumerate(_t_strip()):
        if ent is None:
            continue
        for src in _slot_src(ent, 1):
            CM[src, c0 + s] += 1.0
    plans3.append(dict(
        stat=(32, c0, 32, 32), mov=32, K=32,
        out=("T", 0, 96, 32), tp=(32, 96),
        start=True, stop=True, tag="mm3T4",
    ))

    # --- mm4 vec: one K=96 instr per F strip, T/T ---
    plans4 = []
    for bank in range(4):
        c0 = alloc(32)
        for strip in range(3):
            lay = _bank_strip(bank, strip)
            p0 = _GB[bank][0]
            for s, ent in enumerate(lay):
                if ent is None:
                    continue
                CM[32 * strip + s, c0 + (ent[2] - p0)] = 0.5
        plans4.append(dict(
            stat=(0, c0, 96, 32), movbank=bank, mov=0, K=96,
            out=(32 * bank, 32), tp=(0, 32 * bank),
            start=True, stop=True, tag=f"mm4F{bank}",
        ))

    # --- mm4 tens: same-tp chain K=32 @ (96, 32j) ---
    # contributors: V-bank strip3s (t_d0..t_d3) then T-bank d4
    g_plans = []
    for ci in range(5):
        c0 = alloc(32)
        lay = _t_strip() if ci == 4 else _bank_strip(ci, 3)
        for s, ent in enumerate(lay):
            if ent is None:
                continue
            CM[96 + s, c0 + ent[2]] = 0.5
        g_plans.append(dict(
            stat=(96, c0, 32, 32), movbank=("T" if ci == 4 else ci),
            mov=96, K=32, M=32, tag=f"mm4G{ci}",
        ))

    assert col[0] <= W, col[0]
    return CM, dict(s1=s1_ref, e4=e4_refs, mm3=plans3, mm4=plans4,
                    mm4g=g_plans)


# ---------------------------------------------------------------------------
# numpy reference of the device pipeline (for sim testing)
# ---------------------------------------------------------------------------

def _np_forward(hT):
    """hT [112, n] -> (outD [128,n], outG [128,n/4], outN [128,n/4])"""
    x = np.asarray(hT, np.float32)
    n = x.shape[1]
    sq = x * x
    n2 = np.zeros((24, n), np.float32)
    for i in range(NV):
        n2[i] = sq[_vrow(0, i)] + sq[_vrow(1, i)] + sq[_vrow(2, i)]
    for a in range(NT):
        n2[16 + a] = (sq[_TBASE[0] + a] + sq[_TBASE[1] + a]
                      + sq[_TBASE[2] + a] + sq[_TBASE[3] + a]
                      + sq[_T4BASE[1] + a])
    rinv = 1.0 / np.sqrt(n2 + EPS2)
    norm = n2 * rinv
    rexp = np.zeros_like(x)
    for i in range(NV):
        for r in (_vrow(0, i), _vrow(1, i), _vrow(2, i)):
            rexp[r] = rinv[i]
    for a in range(NT):
        for d in range(4):
            rexp[_TBASE[d] + a] = rinv[16 + a]
        for s in range(4):
            rexp[_T4BASE[s] + a] = rinv[16 + a]
    vu = x * rexp

    def pair_sum(ent, rg):
        r1, r2 = _slot_src(ent, rg)
        return vu[r1] + vu[r2]

    # squares per bank slot
    squ = {}
    for bank in range(4):
        sb = np.zeros((128, n), np.float32)
        for strip in range(4):
            rg = ((strip - bank) % 3) if strip < 3 else bank
            for s, ent in enumerate(_bank_strip(bank, strip)):
                if ent is None:
                    continue
                u = pair_sum(ent, rg)
                sb[32 * strip + s] = u * u
        squ[bank] = sb
    sT = np.zeros((128, n), np.float32)
    for s, ent in enumerate(_t_strip()):
        if ent is None:
            continue
        u = pair_sum(ent, 1)
        sT[96 + s] = u * u

    F = np.full((128, n), -1.0, np.float32)
    for bank in range(4):
        p0 = _GB[bank][0]
        for strip in range(3):
            for s, ent in enumerate(_bank_strip(bank, strip)):
                if ent is None:
                    continue
                F[32 * bank + (ent[2] - p0)] += 0.5 * squ[bank][32 * strip + s]

    ng = n // (4 * CHUNK)
    G = np.full((128, ng * CHUNK), -1.0, np.float32)
    N = np.zeros((128, ng * CHUNK), np.float32)
    for g in range(ng):
        for j in range(4):
            c = 4 * g + j
            cols = slice(c * CHUNK, (c + 1) * CHUNK)
            gcols = slice(g * CHUNK, (g + 1) * CHUNK)
            acc = np.full((32, CHUNK), -1.0, np.float32)
            for bank in range(4):
                for s, ent in enumerate(_bank_strip(bank, 3)):
                    if ent is None:
                        continue
                    acc[ent[2]] += 0.5 * squ[bank][96 + s, cols]
            for s, ent in enumerate(_t_strip()):
                if ent is None:
                    continue
                acc[ent[2]] += 0.5 * sT[96 + s, cols]
            G[32 * j:32 * j + 32, gcols] = acc
            N[32 * j:32 * j + 24, gcols] = norm[:, cols]
    return F, G, N


# ---------------------------------------------------------------------------
# bass kernel
# ---------------------------------------------------------------------------

def _build_nc(n_chunks=N_CHUNKS, padded=PADDED):
    import concourse.bacc as bacc
    import concourse.bass as bass
    import concourse.tile as tile
    from concourse import mybir

    ACT = mybir.ActivationFunctionType
    f32, bf16 = mybir.dt.float32, mybir.dt.bfloat16

    import concourse.hw_specs as hw_specs

    if not getattr(hw_specs, "_invx_patched", False):
        _orig_tables = hw_specs.get_activation_tables

        def _only_nle(module_arch):
            tabs = _orig_tables(module_arch)
            keep = "natural_log_exp_and_others"
            assert keep in tabs
            return {
                name: (funcs if name == keep else set())
                for name, funcs in tabs.items()
            }

        hw_specs.get_activation_tables = _only_nle
        import concourse.bacc as _bacc_mod

        _bacc_mod.get_activation_tables = _only_nle
        hw_specs._invx_patched = True

    CM, plans = _build_consts()
    CW = CM.shape[1]
    use_gps = not os.environ.get("KV3_NOGPS")

    nc = bacc.Bacc("TRN2", target_bir_lowering=False, debug=False,
                   num_devices=N_CORES)

    eps_t = nc.alloc_sbuf_tensor("const-f32-eps2", [128, 1], f32)
    nc.gpsimd.memset(eps_t.ap(), EPS2)
    nc.const_aps.aps[(f32, EPS2)] = eps_t.ap()
    nc.all_engine_barrier()

    ht_ext = nc.declare_dram_parameter("hT", [NIN, padded], bf16, isOutput=False)
    cm_ext = nc.declare_dram_parameter("CM", [128, CW], bf16, isOutput=False)
    n_groups = (n_chunks + 3) // 4
    outD_ext = nc.declare_dram_parameter("outD", [128, padded], bf16, isOutput=True)
    outG_ext = nc.declare_dram_parameter("outG", [128, n_groups * CHUNK], bf16, isOutput=True)
    outN_ext = nc.declare_dram_parameter("outN", [128, n_groups * CHUNK], bf16, isOutput=True)

    with tile.TileContext(nc) as tc:
        with (
            tc.tile_pool(name="const", bufs=1) as cpool,
            tc.tile_pool(name="x", bufs=10) as xpool,
            tc.tile_pool(name="sq", bufs=4) as sqpool,
            tc.tile_pool(name="grp", bufs=2) as grppool,
            tc.tile_pool(name="vu", bufs=3) as vupool,
            tc.tile_pool(name="squ", bufs=2) as squpool,
            tc.tile_pool(name="tsb", bufs=2) as tsbpool,
            tc.tile_pool(name="oa", bufs=3) as oapool,
            tc.tile_pool(name="og", bufs=2) as ogpool,
            tc.tile_pool(name="ps_n2", bufs=1, space=bass.MemorySpace.PSUM) as ps_n2,
            tc.tile_pool(name="ps_u01", bufs=1, space=bass.MemorySpace.PSUM) as ps_u01,
            tc.tile_pool(name="ps_u23", bufs=1, space=bass.MemorySpace.PSUM) as ps_u23,
            tc.tile_pool(name="ps_T", bufs=1, space=bass.MemorySpace.PSUM) as ps_T,
            tc.tile_pool(name="ps_fr", bufs=1, space=bass.MemorySpace.PSUM) as ps_fr,
            tc.tile_pool(name="ps_G", bufs=1, space=bass.MemorySpace.PSUM) as ps_G,
        ):
            cm_t = cpool.tile([128, CW], bf16)
            nc.sync.dma_start(out=cm_t[:], in_=cm_ext[:])

            def stat_ap(ref):
                r0, c0, K, M = ref
                return cm_t[r0:r0 + K, c0:c0 + M]

            for g in range(n_groups):
                chunks = list(range(4 * g, min(4 * g + 4, n_chunks)))

                n2g = ps_n2.tile([128, CHUNK], f32, tag="n2g")
                xs = {}
                for c in chunks:
                    j = c % 4
                    x_t = xpool.tile([NIN, CHUNK], bf16, tag="x")
                    nc.sync.dma_start(
                        out=x_t[:], in_=ht_ext[:, c * CHUNK:(c + 1) * CHUNK])
                    xs[c] = x_t
                    sq_t = sqpool.tile([NIN, CHUNK], bf16, tag="sq")
                    if use_gps:
                        nc.gpsimd.tensor_mul(sq_t[:], x_t[:], x_t[:])
                    else:
                        nc.vector.tensor_mul(sq_t[:], x_t[:], x_t[:])
                    nc.tensor.matmul(
                        n2g[32 * j:32 * j + 32, :], stat_ap(plans["s1"]),
                        sq_t[:], tile_position=(0, 32 * j))

                # group norm path: rinv = exp(-0.5 ln(n2+eps))
                lng = grppool.tile([128, CHUNK], f32, tag="lng")
                nc.scalar.activation(lng[:], n2g[:], ACT.Ln,
                                     bias=EPS2, scale=1.0)
                rinvg = grppool.tile([128, CHUNK], bf16, tag="rinvg")
                nc.scalar.activation(rinvg[:], lng[:], ACT.Exp,
                                     bias=0.0, scale=-0.5)
                normn = grppool.tile([128, CHUNK], bf16, tag="normn")
                nc.vector.tensor_mul(normn[:], n2g[:], rinvg[:])
                gcols = slice(g * CHUNK, (g + 1) * CHUNK)
                nc.sync.dma_start(out=outN_ext[:, gcols], in_=normn[:])

                gdone = ps_G.tile([128, CHUNK], f32, tag="G")

                # rexp lives in the u23 bank (free until this chunk's mm3):
                # mm2_{c+1} only waits on square(u23_c), not on evacF.
                def emit_mm2(c, u23_t):
                    j = c % 4
                    nc.tensor.matmul(
                        u23_t[0:NIN, 0:CHUNK], stat_ap(plans["e4"][j]),
                        rinvg[32 * j:32 * j + 24, :], tile_position=(32 * j, 0))

                def emit_vu(c, u23_t):
                    vu_t = vupool.tile([NIN, CHUNK], bf16, tag="vu")
                    nc.vector.tensor_mul(vu_t[:], xs[c][:], u23_t[0:NIN, 0:CHUNK])
                    return vu_t

                u23_next = ps_u23.tile([128, 2 * CHUNK], f32, tag="u23")
                emit_mm2(chunks[0], u23_next)
                vu_next = emit_vu(chunks[0], u23_next)

                for ci, c in enumerate(chunks):
                    j = c % 4
                    vu_t = vu_next
                    u23 = u23_next

                    u01 = ps_u01.tile([128, 2 * CHUNK], f32, tag="u01")
                    uT = ps_T.tile([128, CHUNK], f32, tag="uT")

                    def u_ap(bank, p0, M):
                        if bank == "T":
                            return uT[p0:p0 + M, :]
                        t = u01 if bank < 2 else u23
                        off = 0 if bank % 2 == 0 else CHUNK
                        return t[p0:p0 + M, off:off + CHUNK]

                    for p in plans["mm3"]:
                        kind, bank, p0, M = p["out"]
                        nc.tensor.matmul(
                            u_ap(bank if kind == "V" else "T", p0, M),
                            stat_ap(p["stat"]),
                            vu_t[p["mov"]:p["mov"] + p["K"], :],
                            tile_position=p["tp"],
                            start=p["start"], stop=p["stop"])

                    # squares
                    s01 = squpool.tile([128, 2 * CHUNK], bf16, tag="s01")
                    nc.scalar.activation(s01[:], u01[:], ACT.Square,
                                         bias=0.0, scale=1.0)
                    s23 = squpool.tile([128, 2 * CHUNK], bf16, tag="s23")
                    nc.scalar.activation(s23[:], u23[:], ACT.Square,
                                         bias=0.0, scale=1.0)
                    # T bank: copy d4 u out of PSUM (rows 96:128), square on
                    # gpsimd (keeps ACT off the critical path)
                    tsb = tsbpool.tile([128, CHUNK], bf16, tag="tsb")
                    nc.vector.tensor_scalar(
                        tsb[96:128, :], uT[96:128, :], 1.0, 0.0,
                        mybir.AluOpType.mult, mybir.AluOpType.add)
                    sT = tsbpool.tile([128, CHUNK], bf16, tag="sT")
                    if use_gps:
                        nc.gpsimd.tensor_mul(sT[96:128, :], tsb[96:128, :],
                                             tsb[96:128, :])
                    else:
                        nc.vector.tensor_mul(sT[96:128, :], tsb[96:128, :],
                                             tsb[96:128, :])

                    def squ_ap(bank, r0, K):
                        if bank == "T":
                            return sT[r0:r0 + K, :]
                        t = s01 if bank < 2 else s23
                        off = 0 if bank % 2 == 0 else CHUNK
                        return t[r0:r0 + K, off:off + CHUNK]

                    # next chunk's mm2 (PE) + vu (DVE) can start as soon as
                    # square(u23_c) has drained the shared bank
                    if ci + 1 < len(chunks):
                        u23_next = ps_u23.tile([128, 2 * CHUNK], f32,
                                               tag="u23", name="u23_next")
                        emit_mm2(chunks[ci + 1], u23_next)

                    # mm4 vec: one instr per F strip, no accumulation
                    F = ps_fr.tile([128, CHUNK], f32, tag="F")
                    for p in plans["mm4"]:
                        p0, M = p["out"]
                        nc.tensor.matmul(
                            F[p0:p0 + M, :], stat_ap(p["stat"]),
                            squ_ap(p["movbank"], p["mov"], p["K"]),
                            tile_position=p["tp"],
                            start=p["start"], stop=p["stop"])

                    # mm4 tens: same-tp chain into G strip j
                    ngp = len(plans["mm4g"])
                    for gi, p in enumerate(plans["mm4g"]):
                        nc.tensor.matmul(
                            gdone[32 * j:32 * j + 32, :], stat_ap(p["stat"]),
                            squ_ap(p["movbank"], p["mov"], p["K"]),
                            tile_position=(96, 32 * j),
                            start=(gi == 0), stop=(gi == ngp - 1))

                    # evac F
                    oa = oapool.tile([128, CHUNK], bf16, tag="oa")
                    nc.vector.tensor_scalar(
                        oa[:], F[:], 1.0, -1.0,
                        mybir.AluOpType.mult, mybir.AluOpType.add)
                    nc.sync.dma_start(
                        out=outD_ext[:, c * CHUNK:(c + 1) * CHUNK], in_=oa[:])

                    if ci + 1 < len(chunks):
                        vu_next = emit_vu(chunks[ci + 1], u23_next)

                # evac G
                og = ogpool.tile([128, CHUNK], bf16, tag="og")
                nc.vector.tensor_scalar(
                    og[:], gdone[:], 1.0, -1.0,
                    mybir.AluOpType.mult, mybir.AluOpType.add)
                nc.sync.dma_start(out=outG_ext[:, gcols], in_=og[:])

    nc.compile()
    return nc


def _get_nc():
    if "nc" not in _CACHE:
        _CACHE["nc"] = _build_nc()
    return _CACHE["nc"]


# ---------------------------------------------------------------------------
# host prep / assembly
# ---------------------------------------------------------------------------

def _make_hT(shard, padded=PADDED):
    """shard [n, 120] f32 -> [112, padded] bf16 (d-major, t_d4 replicated)."""
    n = shard.shape[0]
    rows = _in_rowmap()
    buf = np.ones((padded, NIN), np.float32)
    buf[:n] = shard[:, rows]
    return np.ascontiguousarray(buf.T).astype(BF16)


def _prep_in_maps(h):
    cm = _build_consts()[0].astype(BF16)
    return [
        {"hT": _make_hT(h[c * PER_CORE:(c + 1) * PER_CORE]), "CM": cm}
        for c in range(N_CORES)
    ]


def _assemble(res, n=PER_CORE):
    """device outs -> [n, 204] reference layout (f32)."""
    outD = np.asarray(res["outD"], np.float32)
    outG = np.asarray(res["outG"], np.float32)
    outN = np.asarray(res["outN"], np.float32)
    o = np.empty((n, NOUT), np.float32)
    ng = outN.shape[1] // CHUNK
    vt = np.empty((24, outD.shape[1]), np.float32)
    gt = np.empty((28, outD.shape[1]), np.float32)
    for g in range(ng):
        for j in range(4):
            c = 4 * g + j
            if c * CHUNK >= outD.shape[1]:
                break
            cols = slice(c * CHUNK, (c + 1) * CHUNK)
            gcols = slice(g * CHUNK, (g + 1) * CHUNK)
            vt[:, cols] = outN[32 * j:32 * j + 24, gcols]
            gt[:, cols] = outG[32 * j:32 * j + 28, gcols]
    o[:, 32:48] = vt[0:16, :n].T
    o[:, 48:56] = vt[16:24, :n].T
    o[:, 56:176] = outD[0:120, :n].T
    o[:, 176:204] = gt[:, :n].T
    return o


def _assemble_all(res_list, h):
    out = np.empty((N_ATOMS, NOUT), np.float32)
    out[:, 0:32] = h[:, 0:32]
    for c in range(N_CORES):
        out[c * PER_CORE:(c + 1) * PER_CORE, 32:] = _assemble(res_list[c])[:, 32:]
    return out


# ---------------------------------------------------------------------------
# PJRT runner
# ---------------------------------------------------------------------------

def _run_pjrt(nc, in_maps):
    import jax
    from jax.sharding import Mesh, NamedSharding, PartitionSpec
    from jax.experimental.shard_map import shard_map
    from concourse import mybir
    from concourse.bass2jax import (
        _bass_exec_p,
        install_neuronx_cc_hook,
        partition_id_tensor,
    )

    install_neuronx_cc_hook()
    partition_name = nc.partition_id_tensor.name if nc.partition_id_tensor else None
    in_names, out_names, out_avals = [], [], []
    for alloc in nc.m.functions[0].allocations:
        if not isinstance(alloc, mybir.MemoryLocationSet):
            continue
        name = alloc.memorylocations[0].name
        if alloc.kind == "ExternalInput":
            if name != partition_name:
                in_names.append(name)
        elif alloc.kind == "ExternalOutput":
            out_names.append(name)
            shape = tuple(alloc.tensor_shape)
            dtype = mybir.dt.np(alloc.dtype)
            out_avals.append(jax.core.ShapedArray(shape, dtype))
    n_params = len(in_names)
    n_outs = len(out_avals)
    all_in_names = list(in_names) + out_names
    if partition_name is not None:
        all_in_names.append(partition_name)
    donate = tuple(range(n_params, n_params + n_outs))

    def _body(*args):
        operands = list(args)
        if partition_name is not None:
            operands.append(partition_id_tensor())
        outs = _bass_exec_p.bind(
            *operands,
            out_avals=tuple(out_avals),
            in_names=tuple(all_in_names),
            out_names=tuple(out_names),
            lowering_input_output_aliases=(),
            sim_require_finite=True,
            sim_require_nnan=True,
            nc=nc,
        )
        return tuple(outs)

    devices = jax.devices()[:N_CORES]
    mesh = Mesh(np.asarray(devices), ("core",))
    sharding = NamedSharding(mesh, PartitionSpec("core"))
    fn = jax.jit(
        shard_map(
            _body,
            mesh=mesh,
            in_specs=(PartitionSpec("core"),) * (n_params + n_outs),
            out_specs=(PartitionSpec("core"),) * n_outs,
            check_rep=False,
        ),
        donate_argnums=donate,
        keep_unused=True,
    )

    def make_global(per_core_arrays):
        a0 = per_core_arrays[0]
        gshape = (N_CORES * a0.shape[0],) + a0.shape[1:]
        bufs = [
            jax.device_put(per_core_arrays[c], devices[c]) for c in range(N_CORES)
        ]
        return jax.make_array_from_single_device_arrays(gshape, sharding, bufs)

    g_ins = [
        make_global([np.asarray(in_maps[c][nm]) for c in range(N_CORES)])
        for nm in in_names
    ]
    g_zeros = [
        make_global([np.zeros(av.shape, av.dtype) for _ in range(N_CORES)])
        for av in out_avals
    ]
    outs = fn(*g_ins, *g_zeros)
    jax.block_until_ready(outs)

    results = [dict() for _ in range(N_CORES)]
    for i, nm in enumerate(out_names):
        shards = sorted(
            outs[i].addressable_shards, key=lambda s: devices.index(s.device)
        )
        for c, sh in enumerate(shards):
            results[c][nm] = np.asarray(sh.data)
    return results


def kernel(h):
    h = np.asarray(h, dtype=np.float32)
    assert h.shape == (N_ATOMS, 120)
    nc = _get_nc()
    in_maps = _prep_in_maps(h)
    res = _run_pjrt(nc, in_maps)
    return _assemble_all(res, h)
